# Initial kernel scaffold
#
"""Trainium2 Bass kernel for a 4-layer pre-LN transformer + GEGLU FFN.

Sharding: rows (batch*seq) split across 8 cores; cores 0-3 own batch 0,
cores 4-7 own batch 1 (512 rows each).  Attention needs full-sequence K/V
per batch element, so each 4-core group AllGathers its K/V shards per layer
(two fp8 collectives: K 256KB, then V 256KB without the ones padding).

fp8 (e4m3) on all matmul paths:
  LN (stats on DVE, rstd via fast-reciprocal + ACT sqrt) writes h bf16;
  x-bar DMA transpose to h^T; fp8 cast on ACT.  All projections
  (QKV/O/GEGLU/FFN-out) are fp8 DoubleRow matmuls (256-wide contraction,
  2 accumulation steps, chunk-paired operands; dual-fp8 ldweights needs
  all 128 PE columns + legal AP shapes).  Scores are plain fp8 matmuls.
  exp on ACT writes esb bf16 (fp8 esb makes exp 20% slower); AV is
  mixed fp8-V x bf16-esb with lhsT = [ones | V_h]: av partitions 0-63
  get the softmax sums, 64-127 o^T.  Normalize: reciprocal_approx_fast
  + GpSimd partition_broadcast (both require base partition 0) + one
  DVE multiply.  Out-projection + residual is fused per row-tile with
  the next layer's LN; K^T is produced and shipped per k-block so the
  AllGather triggers early; next-layer weights prefetch during the head
  loop; the last head's AV first half overlaps its own tail exps.
"""

import numpy as np
import ml_dtypes

B, S, C = 2, 2048, 512
L, H, CH = 4, 8, 64
OD = 4 * CH  # 256
EPS = 1e-5

N_CORES = 8
GROUP = 4          # cores per batch element
ROWS = (B * S) // N_CORES  # 512 rows per core
P = 128
RT = ROWS // P     # 4 row tiles
CCH = C // P       # 4 chunks of the hidden/attention dim
JC = 2             # DoubleRow chunk pairs over C (2 x 256)
KT = S // P        # 16 k tiles (full sequence)
KTO = ROWS // P    # 4 own k tiles
SQRT_K = 0x5F3759DF

BF16 = ml_dtypes.bfloat16
F8 = ml_dtypes.float8_e4m3fn

_CACHE = {}


def _f8(w):
    return np.clip(np.asarray(w, np.float32), -240, 240).astype(F8)


def _pack_T(w):
    """Stationary weights for transposed-output projections (Wq/Wk/Wg):
    [C, N] -> [128, (j, mc, s, m)] with row c = (2j+s)*128 + p, so each
    (mc, j) slice is a contiguous [p, 2, 128] dual-fp8 weight block."""
    n = w.shape[1]
    mc = n // P
    return np.ascontiguousarray(
        w.reshape(JC, 2, P, mc, P).transpose(2, 0, 3, 1, 4).reshape(P, -1)
    )


def _pack_CP(w):
    """Moving weights for chunk-paired contraction (Wo):
    [C, N] -> [128, (j, s, n)] with row c = (2j+s)*128 + p."""
    n = w.shape[1]
    return np.ascontiguousarray(
        w.reshape(JC, 2, P, n).transpose(2, 0, 1, 3).reshape(P, -1)
    )


def _pack_WF(w):
    """Wf [256, N] -> [128, (s, n)] with row = s*128 + p."""
    n = w.shape[1]
    return np.ascontiguousarray(
        w.reshape(2, P, n).transpose(1, 0, 2).reshape(P, -1)
    )


def _build(flags, n_layers=L, debug=False):
    use_gamma, use_beta, use_bo, use_bg, use_bf = flags
    import concourse.bass as bass
    import concourse.bacc as bacc
    import concourse.mybir as mybir
    import concourse.tile as tile

    dt = mybir.dt
    OP = mybir.AluOpType
    DR = mybir.MatmulPerfMode.DoubleRow

    nc = bacc.Bacc("TRN2", target_bir_lowering=False, debug=False,
                   num_devices=N_CORES)
    groups = [list(range(g * GROUP, (g + 1) * GROUP))
              for g in range(N_CORES // GROUP)]

    # ---- DRAM I/O ----
    x_d = nc.dram_tensor("x", [P, RT * C], dt.float32, kind="ExternalInput")
    wq_d = nc.dram_tensor("wq", [L, P, CCH * C], dt.float8e4, kind="ExternalInput")
    wk_d = nc.dram_tensor("wk", [L, P, CCH * C], dt.float8e4, kind="ExternalInput")
    wv_d = nc.dram_tensor("wv", [L, P, CCH * C], dt.float8e4, kind="ExternalInput")
    wo_d = nc.dram_tensor("wo", [L, P, CCH * C], dt.float8e4, kind="ExternalInput")
    wg_d = nc.dram_tensor("wg", [P, CCH * C], dt.float8e4, kind="ExternalInput")
    wf_d = nc.dram_tensor("wf", [P, 2 * C], dt.float8e4, kind="ExternalInput")
    y_d = nc.dram_tensor("y", [ROWS, C], dt.float32, kind="ExternalOutput")
    if use_gamma:
        gam_d = nc.dram_tensor("gam", [L + 1, P, C], dt.float32, kind="ExternalInput")
    if use_beta:
        bet_d = nc.dram_tensor("bet", [L + 1, P, C], dt.float32, kind="ExternalInput")
    if use_bo:
        bo_d = nc.dram_tensor("bob", [L, P, C], dt.float32, kind="ExternalInput")
    if use_bg:
        bg_d = nc.dram_tensor("bgc", [P, CCH], dt.float32, kind="ExternalInput")
    if use_bf:
        bf_d = nc.dram_tensor("bfb", [P, C], dt.float32, kind="ExternalInput")

    # split K / V bounce buffers: the K AllGather completes (and unblocks
    # score matmuls) while V's runs; V ships without the ones padding
    kin_k = [nc.dram_tensor(f"kin_k{i}", [P, CCH * ROWS], dt.float8e4,
                            kind="Internal") for i in range(2)]
    kout_k = [nc.dram_tensor(f"kout_k{i}", [GROUP, P, CCH * ROWS], dt.float8e4,
                             kind="Internal") for i in range(2)]
    kin_v = [nc.dram_tensor(f"kin_v{i}", [P, H * KTO * CH], dt.float8e4,
                            kind="Internal") for i in range(2)]
    kout_v = [nc.dram_tensor(f"kout_v{i}", [GROUP, P, H * KTO * CH], dt.float8e4,
                             kind="Internal") for i in range(2)]
    wu_in = nc.dram_tensor("wu_in", [P, 8], dt.float8e4, kind="Internal")
    dbg = {}
    if debug:
        for nm, w, dtt in [("hsb", RT * C, dt.bfloat16), ("ht8", CCH * ROWS, dt.float8e4),
                           ("qt8", CCH * ROWS, dt.float8e4), ("ks8", CCH * ROWS, dt.float8e4),
                           ("kt8", CCH * S, dt.float8e4), ("vst", H * KTO * P, dt.float8e4),
                           ("vs8", H * KT * P, dt.float8e4), ("esb0", KT * ROWS, dt.float8e4),
                           ("av0", ROWS, dt.float32), ("osc", CCH * ROWS, dt.float8e4),
                           ("xsb2", RT * C, dt.float32)]:
            dbg[nm] = nc.dram_tensor("dbg_" + nm, [P, w], dtt, kind="ExternalOutput")
    wu_out = nc.dram_tensor("wu_out", [GROUP, P, 8], dt.float8e4,
                            kind="Internal")

    # ---- persistent SBUF ----
    XSB = nc.alloc_sbuf_tensor("xsb", [P, RT * C], dt.float32).ap()
    HSB = nc.alloc_sbuf_tensor("hsb", [P, RT * C], dt.bfloat16).ap()
    HTB = nc.alloc_sbuf_tensor("htb", [P, CCH * ROWS], dt.bfloat16).ap()
    HT8 = nc.alloc_sbuf_tensor("ht8", [P, CCH * ROWS], dt.float8e4).ap()
    QT8 = nc.alloc_sbuf_tensor("qt8", [P, CCH * ROWS], dt.float8e4).ap()
    KS8 = nc.alloc_sbuf_tensor("ks8", [P, CCH * ROWS], dt.float8e4).ap()
    KT8 = nc.alloc_sbuf_tensor("kt8", [P, CCH * S], dt.float8e4).ap()
    VST = nc.alloc_sbuf_tensor("vst", [P, H * KTO * CH], dt.float8e4).ap()
    VS8 = nc.alloc_sbuf_tensor("vs8", [P, H * KT * P], dt.float8e4).ap()
    OSC = nc.alloc_sbuf_tensor("osc", [P, CCH * ROWS], dt.float8e4).ap()
    FFSB = nc.alloc_sbuf_tensor("ffsb", [P, 2 * ROWS], dt.float8e4).ap()
    RECF = nc.alloc_sbuf_tensor("recf", [P, 2 * ROWS], dt.float32).ap()
    RB = nc.alloc_sbuf_tensor("rb", [P, 2 * ROWS], dt.float32).ap()

    htb_v = HTB.rearrange("p (cc r) -> p cc r", cc=CCH)
    ht8c = HT8.rearrange("p (cc r) -> p cc r", cc=CCH)
    vv_st = VST.rearrange("p (h kt c) -> p h kt c", h=H, kt=KTO)  # c = CH
    vv = VS8.rearrange("p (h kt c) -> p h kt c", h=H, kt=KT)
    ktv = KT8.rearrange("p (cc k) -> p cc k", cc=CCH)
    osc_v = OSC.rearrange("p (cc r) -> p cc r", cc=CCH)
    ff_v = FFSB.rearrange("p (cc r) -> p cc r", cc=2)

    with tile.TileContext(nc) as tc:
        with (
            tc.tile_pool(name="wpool", bufs=2) as wpool,
            tc.tile_pool(name="epool", bufs=5) as epool,
            tc.tile_pool(name="small", bufs=2) as small,
            tc.tile_pool(name="gpool", bufs=2) as gpool,
            tc.tile_pool(name="mmps", bufs=2, space="PSUM") as mmps,
            tc.tile_pool(name="scps", bufs=3, space="PSUM") as scps,
        ):
            # one-time init
            nc.gpsimd.memset(vv[:, :, :, 0:CH], 1.0)
            nc.sync.dma_start(XSB, x_d.ap())

            def ln_chain(li, outproj=None):
                """LN of XSB -> h bf16 (HSB) -> h^T (HTB) -> fp8 (HT8).
                If outproj=(wo_v, BO) is given, each row-tile's out-proj +
                residual add runs immediately before its LN stats so the
                epilogue chain pipelines."""
                MV = small.tile([P, 2 * RT], dt.float32, tag="mv")
                for rt in range(RT):
                    if outproj is not None:
                        wo_v, BO = outproj
                        ps = mmps.tile([P, C], dt.float32, tag="mm")
                        for j in range(JC):
                            nc.tensor.matmul(
                                ps[:],
                                lhsT=osc_v[:, 2 * j:2 * j + 2, rt * P:(rt + 1) * P],
                                rhs=wo_v[:, j],
                                start=(j == 0), stop=(j == JC - 1), perf_mode=DR)
                        dst = XSB[:, rt * C:(rt + 1) * C]
                        nc.vector.tensor_add(dst, ps[:], dst)
                        if BO is not None:
                            nc.vector.tensor_add(dst, dst, BO[:])
                    st6 = small.tile([P, 6], dt.float32, tag="st6")
                    nc.vector.bn_stats(st6[:], XSB[:, rt * C:(rt + 1) * C])
                    nc.vector.bn_aggr(MV[:, 2 * rt:2 * rt + 2], st6[:])
                # rstd = sqrt(1/(var+EPS)): custom-DVE fast reciprocal
                # (full-width, base partition 0) + one tiny ACT sqrt
                RS = small.tile([P, RT], dt.float32, tag="rs")
                VT = small.tile([P, RT], dt.float32, tag="vt")
                RV = small.tile([P, RT], dt.float32, tag="rv")
                var = MV[:].rearrange("p (rt two) -> p two rt", two=2)[:, 1, :]
                nc.vector.tensor_scalar(VT[:], var, EPS, None, OP.add)
                nc.vector.reciprocal_approx_fast(RV[:], VT[:])
                nc.scalar.sqrt(RS[:], RV[:])
                r = RS[:]
                if use_gamma:
                    GT = gpool.tile([P, C], dt.float32, tag="gam")
                    nc.sync.dma_start(GT[:], gam_d.ap()[li])
                if use_beta:
                    BT = gpool.tile([P, C], dt.float32, tag="bet")
                    nc.sync.dma_start(BT[:], bet_d.ap()[li])
                ht8r = HT8.rearrange("p (cc k) -> p cc k", cc=CCH)
                htbr = HTB.rearrange("p (cc k) -> p cc k", cc=CCH)
                for rt in range(RT):
                    dst = HSB[:, rt * C:(rt + 1) * C]
                    nc.vector.tensor_scalar(dst, XSB[:, rt * C:(rt + 1) * C],
                                            MV[:, 2 * rt:2 * rt + 1],
                                            r[:, rt:rt + 1],
                                            OP.subtract, OP.mult)
                    if use_gamma:
                        nc.vector.tensor_mul(dst, dst, GT[:])
                    if use_beta:
                        nc.vector.tensor_add(dst, dst, BT[:])
                    nc.sync.dma_start_transpose(
                        htb_v[:, :, rt * P:(rt + 1) * P],
                        HSB[:, rt * C:(rt + 1) * C])
                    # cast this k-block of h^T to fp8 on ACT (idle here)
                    nc.scalar.copy(
                        ht8r[:, :, rt * P:(rt + 1) * P],
                        htbr[:, :, rt * P:(rt + 1) * P])

            def proj_t(wt_v, dst_col):
                """Transposed-output projection via DoubleRow fp8:
                dst[:, mc] = [chan, rows] for each 128-chunk mc."""
                for mc in range(CCH):
                    ps = mmps.tile([P, ROWS], dt.float32, tag="mm")
                    for j in range(JC):
                        nc.tensor.matmul(
                            ps[:], lhsT=wt_v[:, j, mc],
                            rhs=ht8c[:, 2 * j:2 * j + 2, :],
                            start=(j == 0), stop=(j == JC - 1), perf_mode=DR)
                    nc.vector.tensor_copy(dst_col(mc), ps[:])

            def load_w(li):
                WQ = wpool.tile([P, CCH * C], dt.float8e4, tag="wq")
                WK = wpool.tile([P, CCH * C], dt.float8e4, tag="wk")
                WV = wpool.tile([P, CCH * C], dt.float8e4, tag="wv")
                WO = wpool.tile([P, CCH * C], dt.float8e4, tag="wo")
                nc.sync.dma_start(WK[:], wk_d.ap()[li])
                nc.sync.dma_start(WV[:], wv_d.ap()[li])
                nc.sync.dma_start(WQ[:], wq_d.ap()[li])
                nc.sync.dma_start(WO[:], wo_d.ap()[li])
                return (WQ, WK, WV, WO)

            def attn_layer(li, ws):
                WQ, WK, WV, WO = ws
                wq_v = WQ[:].rearrange("p (j mc s m) -> p j mc s m", j=JC, mc=CCH, s=2)
                wk_v = WK[:].rearrange("p (j mc s m) -> p j mc s m", j=JC, mc=CCH, s=2)
                wv_v = WV[:].rearrange("p (j s n) -> p j s n", j=JC, s=2)
                wo_v = WO[:].rearrange("p (j s n) -> p j s n", j=JC, s=2)

                if debug and li == 0:
                    nc.sync.dma_start(dbg["hsb"].ap(), HSB)
                    nc.sync.dma_start(dbg["ht8"].ap(), HT8)

                kin_ki, kout_ki = kin_k[li % 2], kout_k[li % 2]
                kin_vi, kout_vi = kin_v[li % 2], kout_v[li % 2]

                # K^T (own shard), produced per k-block so each block's DMA
                # into the bounce buffer starts as soon as it exists; the
                # AllGather fires after the last block
                ks8r = KS8.rearrange("p (mc k) -> p mc k", mc=CCH)
                kin_r = kin_ki.ap().rearrange("p (mc k) -> p mc k", mc=CCH)
                for rt in range(RT):
                    ps = mmps.tile([P, ROWS], dt.float32, tag="mm")
                    for mc in range(CCH):
                        for j in range(JC):
                            nc.tensor.matmul(
                                ps[:, mc * P:(mc + 1) * P],
                                lhsT=wk_v[:, j, mc],
                                rhs=ht8c[:, 2 * j:2 * j + 2, rt * P:(rt + 1) * P],
                                start=(j == 0), stop=(j == JC - 1), perf_mode=DR)
                    nc.vector.tensor_copy(
                        ks8r[:, :, rt * P:(rt + 1) * P],
                        ps[:].rearrange("p (mc k) -> p mc k", mc=CCH))
                    nc.sync.dma_start(kin_r[:, :, rt * P:(rt + 1) * P],
                                      ks8r[:, :, rt * P:(rt + 1) * P])
                nc.gpsimd.collective_compute(
                    "AllGather", mybir.AluOpType.bypass, replica_groups=groups,
                    ins=[kin_ki.ap().opt()], outs=[kout_ki.ap().opt()])

                # V (own shard, natural layout, no ones in the payload)
                for kt in range(KTO):
                    ps = mmps.tile([P, C], dt.float32, tag="mm")
                    for j in range(JC):
                        nc.tensor.matmul(
                            ps[:],
                            lhsT=ht8c[:, 2 * j:2 * j + 2, kt * P:(kt + 1) * P],
                            rhs=wv_v[:, j],
                            start=(j == 0), stop=(j == JC - 1), perf_mode=DR)
                    nc.vector.tensor_copy(
                        vv_st[:, :, kt, :],
                        ps[:].rearrange("p (h c) -> p h c", h=H))
                nc.sync.dma_start(kin_vi.ap(), VST)
                nc.gpsimd.collective_compute(
                    "AllGather", mybir.AluOpType.bypass, replica_groups=groups,
                    ins=[kin_vi.ap().opt()], outs=[kout_vi.ap().opt()])

                # Q^T (overlaps with the collectives)
                proj_t(wq_v, lambda mc: QT8[:, mc * ROWS:(mc + 1) * ROWS])

                if debug and li == 0:
                    nc.sync.dma_start(dbg["qt8"].ap(), QT8)
                    nc.sync.dma_start(dbg["ks8"].ap(), KS8)
                    nc.sync.dma_start(dbg["vst"].ap(), VST)

                # unload gathered K^T (first; scores need it) then V
                for rr in range(GROUP):
                    nc.sync.dma_start(ktv[:, :, rr * ROWS:(rr + 1) * ROWS],
                                      kout_ki.ap()[rr])
                for rr in range(GROUP):
                    kv_r = kout_vi.ap()[rr].rearrange("p (h kt c) -> p h kt c",
                                                      h=H, kt=KTO)
                    for hh in range(H):
                        nc.sync.dma_start(
                            vv[:, hh, rr * KTO:(rr + 1) * KTO, CH:P],
                            kv_r[:, hh])
                if li + 1 < n_layers:
                    ws_next = load_w(li + 1)
                else:
                    WGp = wpool.tile([P, CCH * C], dt.float8e4, tag="wq")
                    WFp = wpool.tile([P, 2 * C], dt.float8e4, tag="wf")
                    nc.sync.dma_start(WGp[:], wg_d.ap())
                    nc.sync.dma_start(WFp[:], wf_d.ap())
                    ws_next = (WGp, WFp)

                def dbg_dump_kv():
                    nc.sync.dma_start(dbg["kt8"].ap(), KT8)
                    nc.sync.dma_start(dbg["vs8"].ap(), VS8)

                def normalize(h, av):
                    """av = [sums (64 copies) | o^T]: custom-DVE /
                    partition_broadcast only work from partition 0."""
                    sub = (h % 2) * CH
                    cc_h = h // 2
                    slot = h % 2
                    rf = RECF[0:1, slot * ROWS:(slot + 1) * ROWS]
                    nc.vector.reciprocal_approx_fast(rf, av[0:1, :])
                    rb = RB[0:P, slot * ROWS:(slot + 1) * ROWS]
                    nc.gpsimd.partition_broadcast(rb, rf, channels=P)
                    nc.vector.tensor_mul(
                        OSC[sub:sub + CH, cc_h * ROWS:(cc_h + 1) * ROWS],
                        av[CH:P, :],
                        RB[CH:P, slot * ROWS:(slot + 1) * ROWS])

                def emit_av(h, esb, av=None, kt0=0, kt1=KT, fin=True):
                    """AV over k tiles; lhsT = [ones | V_h] (fp8) x bf16 esb,
                    so partitions 0-63 get the softmax sums, 64-127 o^T."""
                    if av is None:
                        av = mmps.tile([P, ROWS], dt.float32, tag="mm")
                    for kt in range(kt0, kt1):
                        nc.tensor.matmul(
                            av[:],
                            lhsT=vv[:, h, kt, :],
                            rhs=esb[:, kt * ROWS:(kt + 1) * ROWS],
                            start=(kt == 0), stop=(kt == KT - 1))
                    if not fin:
                        return av
                    if debug and li == 0 and h == 0:
                        AVD = small.tile([P, ROWS], dt.float32, tag="avd")
                        nc.vector.tensor_copy(AVD[:], av[:])
                        nc.sync.dma_start(dbg["av0"].ap(), AVD[:])
                    normalize(h, av)
                    return av

                # software pipeline: scores/exp of head h overlap the AV
                # matmul chain of head h-1
                pending = None
                for h in range(H):
                    sub = (h % 2) * CH
                    cc_h = h // 2
                    esb = epool.tile([P, KT * ROWS], dt.bfloat16, tag="esb")
                    if debug and li == 0 and h == 0:
                        dbg_dump_kv()
                    for pair in range(KT // 2):
                        sp = scps.tile([P, 2 * ROWS], dt.float32, tag="sc")
                        for j in range(2):
                            kt = 2 * pair + j
                            nc.tensor.matmul(
                                sp[:, j * ROWS:(j + 1) * ROWS],
                                lhsT=KT8[sub:sub + CH,
                                         cc_h * S + kt * P: cc_h * S + (kt + 1) * P],
                                rhs=QT8[sub:sub + CH,
                                        cc_h * ROWS:(cc_h + 1) * ROWS],
                                start=True, stop=True)
                        nc.scalar.activation(
                            esb[:, pair * 2 * ROWS:(pair + 1) * 2 * ROWS],
                            sp[:], mybir.ActivationFunctionType.Exp,
                            scale=1.0 / np.sqrt(CH))
                    if debug and li == 0 and h == 0:
                        nc.sync.dma_start(dbg["esb0"].ap(), esb[:])
                    if pending is not None:
                        emit_av(*pending)
                    if h == H - 1:
                        # last head: first AV half right behind its early exps
                        av_l = emit_av(h, esb, kt0=0, kt1=KT // 2, fin=False)
                    pending = (h, esb)
                emit_av(pending[0], pending[1], av=av_l, kt0=KT // 2)

                # out-projection + residual fused with next LN
                BO = None
                if use_bo:
                    BO = gpool.tile([P, C], dt.float32, tag="bo")
                    nc.sync.dma_start(BO[:], bo_d.ap()[li])
                ln_chain(li + 1, outproj=(wo_v, BO))
                return ws_next

            ln_chain(0)
            ws = load_w(0)
            for li in range(n_layers):
                ws = attn_layer(li, ws)
                if debug and li == 0:
                    nc.sync.dma_start(dbg["osc"].ap(), OSC)
                    nc.sync.dma_start(dbg["xsb2"].ap(), XSB)

            # ---- FFN (LN + weight loads already done by last layer) ----
            WG, WF = ws
            wg_v = WG[:].rearrange("p (j mc s m) -> p j mc s m", j=JC, mc=CCH, s=2)
            wf_v = WF[:].rearrange("p (s n) -> p s n", s=2)
            if use_bg:
                BG = gpool.tile([P, CCH], dt.float32, tag="bg")
                nc.sync.dma_start(BG[:], bg_d.ap())
            AGT = small.tile([P, 2 * ROWS], dt.bfloat16, tag="ffa")
            GGT = small.tile([P, 2 * ROWS], dt.bfloat16, tag="ffg")
            for mg in (0, 2, 1, 3):
                ps = mmps.tile([P, ROWS], dt.float32, tag="mm")
                for j in range(JC):
                    nc.tensor.matmul(
                        ps[:], lhsT=wg_v[:, j, mg], rhs=ht8c[:, 2 * j:2 * j + 2, :],
                        start=(j == 0), stop=(j == JC - 1), perf_mode=DR)
                dst = (AGT if mg < 2 else GGT)[:, (mg % 2) * ROWS:(mg % 2 + 1) * ROWS]
                if use_bg:
                    nc.vector.tensor_scalar(dst, ps[:], BG[:, mg:mg + 1], None,
                                            mybir.AluOpType.add)
                else:
                    nc.vector.tensor_copy(dst, ps[:])
            K1 = 0.7978845608
            for j in range(2):
                ga = GGT[:, j * ROWS:(j + 1) * ROWS]
                aa = AGT[:, j * ROWS:(j + 1) * ROWS]
                SQ = small.tile([P, ROWS], dt.float32, tag="sq")
                WT = small.tile([P, ROWS], dt.float32, tag="wt")
                VV = small.tile([P, ROWS], dt.float32, tag="vv")
                TT = small.tile([P, ROWS], dt.float32, tag="tt")
                HT2 = small.tile([P, ROWS], dt.bfloat16, tag="ht2")
                PP = small.tile([P, ROWS], dt.bfloat16, tag="pp")
                nc.scalar.activation(SQ[:], ga, mybir.ActivationFunctionType.Square)
                nc.vector.tensor_scalar(WT[:], SQ[:], K1 * 0.044715, K1,
                                        mybir.AluOpType.mult, mybir.AluOpType.add)
                nc.vector.tensor_mul(VV[:], ga, WT[:])
                nc.scalar.activation(TT[:], VV[:], mybir.ActivationFunctionType.Tanh)
                nc.vector.tensor_scalar(HT2[:], TT[:], 0.5, 0.5,
                                        mybir.AluOpType.mult, mybir.AluOpType.add)
                nc.vector.tensor_mul(PP[:], aa, ga)
                nc.vector.tensor_mul(FFSB[:, j * ROWS:(j + 1) * ROWS], PP[:], HT2[:])
            if use_bf:
                BF = gpool.tile([P, C], dt.float32, tag="bf")
                nc.sync.dma_start(BF[:], bf_d.ap())
            for rt in range(RT):
                ps = mmps.tile([P, C], dt.float32, tag="mm")
                nc.tensor.matmul(
                    ps[:],
                    lhsT=ff_v[:, :, rt * P:(rt + 1) * P],
                    rhs=wf_v[:],
                    start=True, stop=True, perf_mode=DR)
                OUT = small.tile([P, C], dt.float32, tag="out")
                nc.vector.tensor_add(OUT[:], ps[:], XSB[:, rt * C:(rt + 1) * C])
                if use_bf:
                    nc.vector.tensor_add(OUT[:], OUT[:], BF[:])
                nc.sync.dma_start(y_d.ap()[rt * P:(rt + 1) * P, :], OUT[:])

    nc.compile()
    return nc


def _prepare(x, ln_gamma, ln_beta, Wq, Wk, Wv, Wo, bo, Wg, bg, Wf, bf):
    """Host-side packing: returns (flags, per-core input maps)."""
    x = np.asarray(x, np.float32)
    ln_gamma = np.asarray(ln_gamma, np.float32)
    ln_beta = np.asarray(ln_beta, np.float32)
    Wq, Wk, Wv, Wo = (np.asarray(w, np.float32) for w in (Wq, Wk, Wv, Wo))
    bo = np.asarray(bo, np.float32)
    Wg, Wf = np.asarray(Wg, np.float32), np.asarray(Wf, np.float32)
    bg, bf = np.asarray(bg, np.float32), np.asarray(bf, np.float32)

    use_gamma = not np.all(ln_gamma == 1.0)
    use_beta = not np.all(ln_beta == 0.0)
    use_bo = not np.all(bo == 0.0)
    use_bg = not np.all(bg == 0.0)
    use_bf = not np.all(bf == 0.0)
    flags = (use_gamma, use_beta, use_bo, use_bg, use_bf)

    base = {
        "wq": np.stack([_pack_T(_f8(Wq[l])) for l in range(L)]),
        "wk": np.stack([_pack_T(_f8(Wk[l])) for l in range(L)]),
        "wv": np.stack([_pack_CP(_f8(Wv[l])) for l in range(L)]),
        "wo": np.stack([_pack_CP(_f8(Wo[l])) for l in range(L)]),
        "wg": _pack_T(_f8(Wg)),
        "wf": _pack_WF(_f8(Wf)),
    }
    if use_gamma:
        base["gam"] = np.ascontiguousarray(
            np.broadcast_to(ln_gamma[:, None, :], (L + 1, P, C))).astype(np.float32)
    if use_beta:
        base["bet"] = np.ascontiguousarray(
            np.broadcast_to(ln_beta[:, None, :], (L + 1, P, C))).astype(np.float32)
    if use_bo:
        base["bob"] = np.ascontiguousarray(
            np.broadcast_to(bo[:, None, :], (L, P, C))).astype(np.float32)
    if use_bg:
        base["bgc"] = np.ascontiguousarray(bg.reshape(CCH, P).T).astype(np.float32)
    if use_bf:
        base["bfb"] = np.ascontiguousarray(
            np.broadcast_to(bf[None, :], (P, C))).astype(np.float32)

    xf = x.reshape(B * S, C)
    in_maps = []
    for c in range(N_CORES):
        m = dict(base)
        xc = xf[c * ROWS:(c + 1) * ROWS]
        m["x"] = np.ascontiguousarray(
            xc.reshape(RT, P, C).transpose(1, 0, 2).reshape(P, RT * C))
        in_maps.append(m)
    return flags, in_maps


def kernel(x, ln_gamma, ln_beta, Wq, Wk, Wv, Wo, bo, Wg, bg, Wf, bf):
    flags, in_maps = _prepare(x, ln_gamma, ln_beta, Wq, Wk, Wv, Wo, bo,
                              Wg, bg, Wf, bf)
    if flags not in _CACHE:
        _CACHE[flags] = _build(flags)
    nc = _CACHE[flags]

    from concourse.bass_utils import run_bass_kernel_spmd
    res = run_bass_kernel_spmd(nc, in_maps, core_ids=list(range(N_CORES)))
    out = np.concatenate([res.results[c]["y"] for c in range(N_CORES)], axis=0)
    return out.reshape(B, S, C).astype(np.float32)



# revision 25
# speedup vs baseline: 1.0471x; 1.0471x over previous
"""Trainium2 Bass kernel for a 4-layer pre-LN transformer + GEGLU FFN.

Sharding: rows (batch*seq) split across 8 cores; cores 0-3 own batch 0,
cores 4-7 own batch 1 (512 rows each).  Attention needs full-sequence K/V
per batch element, so each 4-core group AllGathers its K/V shards per layer.

v4 design — query-half software pipeline, ACT(exp)-bound schedule:
  Attention is embarrassingly parallel across queries, so each layer is
  split into two query-halves A (rows 0:256 = rts 0,1) and B (256:512).
  A half's tail chain (out-proj -> residual -> next-layer LN -> K^T/V/Q^T
  projections -> AllGather trigger -> unload) runs on PE/DVE/DMA *under*
  the other half's exp stream, hiding both the serial chain and the
  collective latency that previously cost ~50us per layer boundary.

  ACT stream per layer: evensA | evensB | oddsA | oddsB (64 exps of
  [128,1024], fp32 psum -> fp8 esb).  gather[0] (K+V of rows rt0,1 from
  every group rank = even key-chunks {2rr}) is triggered by tail(li-1,A)
  and lands during oddsB(li-1)/evensA; gather[1] (odd chunks) triggered
  by tail(li-1,B) lands during evensA/evensB.

  - exp writes esb fp8 directly (measured: same ACT speed as bf16 out),
    enabling DoubleRow AV: 8 accumulation steps of 256 keys instead of
    16 (HW streams 1 col/cycle regardless of mode; DR halves the
    instruction count of the contraction).
  - KT8 / V stationary double-buffered by layer parity so unloads start
    the moment a gather lands (no WAR on the previous layer's reads).
  - LN rstd on DVE only (magic-number rsqrt + 2 Newton steps): ACT never
    loads another activation table (exp/square/tanh/copy share table 0).
  - PE warmup: dummy matmuls fill the multi-10us core-launch-skew window
    before the first gather so the PE clock (1.2 -> 2.4 GHz DVFS, needs
    ~60us of sustained activity) ramps before the real work.
  - FFN is row-parallel: LN(4) runs in the per-half tails; GEGLU + down
    projection in a short epilogue.

fp8 (e4m3) on all matmul paths; all projections are fp8 DoubleRow.
Scores are plain fp8 matmuls (output-streaming bound).  Softmax sums
come from the 64 ones-rows of the AV output; normalize via DVE
fast-reciprocal + GpSimd partition_broadcast.
"""

import numpy as np
import ml_dtypes

B, S, C = 2, 2048, 512
L, H, CH = 4, 8, 64
OD = 4 * CH  # 256
EPS = 1e-5

N_CORES = 8
GROUP = 4          # cores per batch element
ROWS = (B * S) // N_CORES  # 512 rows per core
P = 128
RT = ROWS // P     # 4 row tiles
RH = ROWS // 2     # 256 rows per query-half
CCH = C // P       # 4 chunks of the hidden/attention dim
JC = 2             # DoubleRow chunk pairs over C (2 x 256)
KT = S // P        # 16 k tiles (full sequence)
NCK = KT // 2      # 8 key chunks of 256 (DR-AV granularity)
MAGIC = 0x5F3759DF
WARMUP_MM = 80

BF16 = ml_dtypes.bfloat16
F8 = ml_dtypes.float8_e4m3fn

_CACHE = {}


def _f8(w):
    return np.clip(np.asarray(w, np.float32), -240, 240).astype(F8)


def _pack_T(w):
    """Stationary weights for transposed-output projections (Wq/Wk/Wg):
    [C, N] -> [128, (j, mc, s, m)] with row c = (2j+s)*128 + p, so each
    (mc, j) slice is a contiguous [p, 2, 128] dual-fp8 weight block."""
    n = w.shape[1]
    mc = n // P
    return np.ascontiguousarray(
        w.reshape(JC, 2, P, mc, P).transpose(2, 0, 3, 1, 4).reshape(P, -1)
    )


def _pack_CP(w):
    """Moving weights for chunk-paired contraction (Wo/Wv):
    [C, N] -> [128, (j, s, n)] with row c = (2j+s)*128 + p."""
    n = w.shape[1]
    return np.ascontiguousarray(
        w.reshape(JC, 2, P, n).transpose(2, 0, 1, 3).reshape(P, -1)
    )


def _pack_WF(w):
    """Wf [256, N] -> [128, (s, n)] with row = s*128 + p."""
    n = w.shape[1]
    return np.ascontiguousarray(
        w.reshape(2, P, n).transpose(1, 0, 2).reshape(P, -1)
    )


# esb column layout: parity-major chunks — chunk c lives at column
# (c%2)*4*RH + (c//2)*2*RH, (sub, 256q) inside a chunk.  A "quad" is two
# same-parity chunks = 4 k-tiles = one [128, 1024] psum/exp block.
EQUADS = [([0, 1, 4, 5], 0), ([8, 9, 12, 13], 4 * RH)]
OQUADS = [([2, 3, 6, 7], 8 * RH), ([10, 11, 14, 15], 12 * RH)]


def _build(flags, n_layers=L):
    use_gamma, use_beta, use_bo, use_bg, use_bf = flags
    import concourse.bass as bass
    import concourse.bacc as bacc
    import concourse.mybir as mybir
    import concourse.tile as tile

    dt = mybir.dt
    OP = mybir.AluOpType
    DR = mybir.MatmulPerfMode.DoubleRow

    nc = bacc.Bacc("TRN2", target_bir_lowering=False, debug=False,
                   num_devices=N_CORES)
    groups = [list(range(g * GROUP, (g + 1) * GROUP))
              for g in range(N_CORES // GROUP)]

    # ---- DRAM I/O ----
    x_d = nc.dram_tensor("x", [P, RT * C], dt.float32, kind="ExternalInput")
    wq_d = nc.dram_tensor("wq", [L, P, CCH * C], dt.float8e4, kind="ExternalInput")
    wk_d = nc.dram_tensor("wk", [L, P, CCH * C], dt.float8e4, kind="ExternalInput")
    wv_d = nc.dram_tensor("wv", [L, P, CCH * C], dt.float8e4, kind="ExternalInput")
    wo_d = nc.dram_tensor("wo", [L, P, CCH * C], dt.float8e4, kind="ExternalInput")
    wg_d = nc.dram_tensor("wg", [P, CCH * C], dt.float8e4, kind="ExternalInput")
    wf_d = nc.dram_tensor("wf", [P, 2 * C], dt.float8e4, kind="ExternalInput")
    y_d = nc.dram_tensor("y", [ROWS, C], dt.float32, kind="ExternalOutput")
    if use_gamma:
        gam_d = nc.dram_tensor("gam", [L + 1, P, C], dt.float32, kind="ExternalInput")
    if use_beta:
        bet_d = nc.dram_tensor("bet", [L + 1, P, C], dt.float32, kind="ExternalInput")
    if use_bo:
        bo_d = nc.dram_tensor("bob", [L, P, C], dt.float32, kind="ExternalInput")
    if use_bg:
        bg_d = nc.dram_tensor("bgc", [P, CCH], dt.float32, kind="ExternalInput")
    if use_bf:
        bf_d = nc.dram_tensor("bfb", [P, C], dt.float32, kind="ExternalInput")

    # Collective bounce buffers: one per (layer parity, half).
    # Payload per half: K^T of 2 row-tiles (mc4, s2, k128) + V (h8, s2, c64).
    KHALF = CCH * 2 * P   # 1024
    VHALF = H * 2 * CH    # 1024
    KV = KHALF + VHALF
    kin = [[nc.dram_tensor(f"kin{i}_{hf}", [P, KV], dt.float8e4,
                           kind="Internal") for hf in range(2)]
           for i in range(2)]
    kout = [[nc.dram_tensor(f"kout{i}_{hf}", [GROUP, P, KV], dt.float8e4,
                            kind="Internal") for hf in range(2)]
            for i in range(2)]

    # ---- persistent SBUF ----
    XSB = nc.alloc_sbuf_tensor("xsb", [P, RT * C], dt.float32).ap()
    HSB = nc.alloc_sbuf_tensor("hsb", [P, RT * C], dt.bfloat16).ap()
    HTB = nc.alloc_sbuf_tensor("htb", [P, CCH * ROWS], dt.bfloat16).ap()
    HT8 = nc.alloc_sbuf_tensor("ht8", [P, CCH * ROWS], dt.float8e4).ap()
    QT8 = nc.alloc_sbuf_tensor("qt8", [P, CCH * ROWS], dt.float8e4).ap()
    KS8 = nc.alloc_sbuf_tensor("ks8", [P, CCH * ROWS], dt.float8e4).ap()
    KT8 = [nc.alloc_sbuf_tensor(f"kt8_{i}", [P, CCH * S], dt.float8e4).ap()
           for i in range(2)]
    VST = nc.alloc_sbuf_tensor("vst", [P, 2 * VHALF], dt.float8e4).ap()
    # DR-AV stationary: [p, h, chunk, sub, m] with m = [64 ones | 64 V-ch]
    VS8 = [nc.alloc_sbuf_tensor(f"vs8_{i}", [P, H * NCK * 2 * P],
                                dt.float8e4).ap() for i in range(2)]
    OSC = nc.alloc_sbuf_tensor("osc", [P, CCH * ROWS], dt.float8e4).ap()
    FFSB = nc.alloc_sbuf_tensor("ffsb", [P, 2 * ROWS], dt.float8e4).ap()
    RECF = nc.alloc_sbuf_tensor("recf", [P, 2 * ROWS], dt.float32).ap()
    RB = nc.alloc_sbuf_tensor("rb", [P, 2 * ROWS], dt.float32).ap()
    # never-written junk operand for warmup / keepalive filler matmuls:
    # reading it creates no dependencies, so fillers slot into PE idle gaps
    DUM = nc.alloc_sbuf_tensor("dum", [P, 1024], dt.float8e4).ap()

    htb_v = HTB.rearrange("p (cc r) -> p cc r", cc=CCH)
    ht8c = HT8.rearrange("p (cc r) -> p cc r", cc=CCH)
    vst_v = VST.rearrange("p (hf h s c) -> p hf h s c", hf=2, h=H, s=2)
    vvs = [VS8[i].rearrange("p (h ck s m) -> p h ck s m", h=H, ck=NCK, s=2)
           for i in range(2)]
    ktvs = [KT8[i].rearrange("p (cc k) -> p cc k", cc=CCH) for i in range(2)]
    osc_v = OSC.rearrange("p (cc r) -> p cc r", cc=CCH)
    ff_v = FFSB.rearrange("p (cc r) -> p cc r", cc=2)
    ks8r = KS8.rearrange("p (mc k) -> p mc k", mc=CCH)

    def esb_off(c):
        return (c % 2) * 4 * RH + (c // 2) * 2 * RH

    with tile.TileContext(nc) as tc:
        with (
            tc.tile_pool(name="wpool", bufs=2) as wpool,
            tc.tile_pool(name="epool", bufs=16) as epool,
            tc.tile_pool(name="small", bufs=2) as small,
            tc.tile_pool(name="gpool", bufs=2) as gpool,
            tc.tile_pool(name="mmps", bufs=2, space="PSUM") as mmps,
            tc.tile_pool(name="avps", bufs=2, space="PSUM") as avps,
            tc.tile_pool(name="scps", bufs=2, space="PSUM") as scps,
        ):
            nc.sync.dma_start(XSB, x_d.ap())
            for i in range(2):
                nc.gpsimd.memset(vvs[i][:, :, :, :, 0:CH], 1.0)

            def fillers(n):
                """No-dependency junk matmuls: PE clock keepalive (the HW
                PE downclocks 2.4->1.2GHz after ~3.5us idle and needs
                ~40us of sustained work to recover)."""
                for i in range(n):
                    wps = mmps.tile([P, ROWS], dt.float32, tag="mm",
                                    name="wup")
                    nc.tensor.matmul(wps[:], lhsT=DUM[0:64, 0:P],
                                     rhs=DUM[0:64, 512:512 + ROWS],
                                     start=True, stop=True)

            # PE warmup through the core-launch-skew window
            fillers(WARMUP_MM)

            def load_w(li):
                WQ = wpool.tile([P, CCH * C], dt.float8e4, tag="wq")
                WK = wpool.tile([P, CCH * C], dt.float8e4, tag="wk")
                WV = wpool.tile([P, CCH * C], dt.float8e4, tag="wv")
                WO = wpool.tile([P, CCH * C], dt.float8e4, tag="wo")
                nc.sync.dma_start(WK[:], wk_d.ap()[li])
                nc.sync.dma_start(WV[:], wv_d.ap()[li])
                nc.sync.dma_start(WQ[:], wq_d.ap()[li])
                nc.sync.dma_start(WO[:], wo_d.ap()[li])
                wq_v = WQ[:].rearrange("p (j mc s m) -> p j mc s m", j=JC, mc=CCH, s=2)
                wk_v = WK[:].rearrange("p (j mc s m) -> p j mc s m", j=JC, mc=CCH, s=2)
                wv_v = WV[:].rearrange("p (j s n) -> p j s n", j=JC, s=2)
                wo_v = WO[:].rearrange("p (j s n) -> p j s n", j=JC, s=2)
                return (wq_v, wk_v, wv_v, wo_v)

            def tail(li, half, wnext, wo_v=None):
                """Row-half tail: out-proj(li) + residual + LN(li+1) for
                rows (2*half, 2*half+1); then K^T/V/Q^T of layer li+1 for
                those rows, ship + AllGather trigger + unload.  Runs on
                PE/DVE/DMA under the other half's exp stream (no ACT)."""
                rts = (2 * half, 2 * half + 1)
                MV = small.tile([P, 4], dt.float32, tag="mv", name="mv")
                for i, rt in enumerate(rts):
                    if wo_v is not None:
                        ps = mmps.tile([P, C], dt.float32, tag="mm", name="op")
                        for j in range(JC):
                            nc.tensor.matmul(
                                ps[:],
                                lhsT=osc_v[:, 2 * j:2 * j + 2, rt * P:(rt + 1) * P],
                                rhs=wo_v[:, j],
                                start=(j == 0), stop=(j == JC - 1), perf_mode=DR)
                        dst = XSB[:, rt * C:(rt + 1) * C]
                        nc.vector.tensor_add(dst, ps[:], dst)
                        if use_bo:
                            BO = gpool.tile([P, C], dt.float32, tag="bo")
                            nc.sync.dma_start(BO[:], bo_d.ap()[li])
                            nc.vector.tensor_add(dst, dst, BO[:])
                    st6 = small.tile([P, 6], dt.float32, tag="st6", name="st6")
                    nc.vector.bn_stats(st6[:], XSB[:, rt * C:(rt + 1) * C])
                    nc.vector.bn_aggr(MV[:, 2 * i:2 * i + 2], st6[:])
                # rstd = rsqrt(var+EPS) on DVE only (no ACT table swap)
                VT = small.tile([P, 2], dt.float32, tag="vt", name="vt")
                T0 = small.tile([P, 2], dt.float32, tag="t0", name="t0")
                RS = small.tile([P, 2], dt.float32, tag="rs", name="rs")
                T1 = small.tile([P, 2], dt.float32, tag="t1", name="t1")
                var = MV[:].rearrange("p (rt two) -> p two rt", two=2)[:, 1, :]
                nc.vector.tensor_scalar(VT[:], var, EPS, None, OP.add)
                nc.vector.tensor_scalar(T0[:].bitcast(dt.int32),
                                        VT[:].bitcast(dt.int32),
                                        1, None, OP.logical_shift_right)
                nc.vector.tensor_scalar(T0[:].bitcast(dt.int32),
                                        T0[:].bitcast(dt.int32),
                                        -1, None, OP.bitwise_xor)
                nc.vector.tensor_scalar(RS[:].bitcast(dt.int32),
                                        T0[:].bitcast(dt.int32),
                                        MAGIC + 1, None, OP.add)
                for _ in range(2):
                    nc.vector.tensor_mul(T1[:], RS[:], RS[:])
                    nc.vector.tensor_mul(T1[:], T1[:], VT[:])
                    nc.vector.tensor_scalar(T1[:], T1[:], -0.5, 1.5,
                                            OP.mult, OP.add)
                    nc.vector.tensor_mul(RS[:], RS[:], T1[:])
                if use_gamma:
                    GT = gpool.tile([P, C], dt.float32, tag="gam")
                    nc.sync.dma_start(GT[:], gam_d.ap()[li + 1])
                if use_beta:
                    BT = gpool.tile([P, C], dt.float32, tag="bet")
                    nc.sync.dma_start(BT[:], bet_d.ap()[li + 1])
                ht8r = HT8.rearrange("p (cc k) -> p cc k", cc=CCH)
                htbr = HTB.rearrange("p (cc k) -> p cc k", cc=CCH)
                for i, rt in enumerate(rts):
                    dst = HSB[:, rt * C:(rt + 1) * C]
                    nc.vector.tensor_scalar(dst, XSB[:, rt * C:(rt + 1) * C],
                                            MV[:, 2 * i:2 * i + 1],
                                            RS[:, i:i + 1],
                                            OP.subtract, OP.mult)
                    if use_gamma:
                        nc.vector.tensor_mul(dst, dst, GT[:])
                    if use_beta:
                        nc.vector.tensor_add(dst, dst, BT[:])
                    nc.sync.dma_start_transpose(
                        htb_v[:, :, rt * P:(rt + 1) * P],
                        HSB[:, rt * C:(rt + 1) * C])
                    # fp8 cast on GpSimd: keeps the tail's serial chain off
                    # the DVE queue, which is busy with normalize work
                    nc.gpsimd.tensor_copy(
                        ht8r[:, :, rt * P:(rt + 1) * P],
                        htbr[:, :, rt * P:(rt + 1) * P])
                if wnext is None:
                    return  # last layer: FFN epilogue reads ht8 directly
                wq_v, wk_v, wv_v, _ = wnext
                par = (li + 1) % 2  # parity of layer li+1
                kin_t, kout_t = kin[par][half], kout[par][half]
                kin_kv = kin_t.ap()[:, 0:KHALF].rearrange(
                    "p (mc s k) -> p mc s k", mc=CCH, s=2)
                # K^T for this half's key rows, per row-tile, ship each
                for i, rt in enumerate(rts):
                    ps = mmps.tile([P, ROWS], dt.float32, tag="mm", name="kp")
                    for mc in range(CCH):
                        for j in range(JC):
                            nc.tensor.matmul(
                                ps[:, mc * P:(mc + 1) * P],
                                lhsT=wk_v[:, j, mc],
                                rhs=ht8c[:, 2 * j:2 * j + 2, rt * P:(rt + 1) * P],
                                start=(j == 0), stop=(j == JC - 1), perf_mode=DR)
                    nc.vector.tensor_copy(
                        ks8r[:, :, rt * P:(rt + 1) * P],
                        ps[:].rearrange("p (mc k) -> p mc k", mc=CCH))
                    nc.sync.dma_start(kin_kv[:, :, i, :],
                                      ks8r[:, :, rt * P:(rt + 1) * P])
                # V for this half's key rows
                for i, rt in enumerate(rts):
                    ps = mmps.tile([P, C], dt.float32, tag="mm", name="vp")
                    for j in range(JC):
                        nc.tensor.matmul(
                            ps[:],
                            lhsT=ht8c[:, 2 * j:2 * j + 2, rt * P:(rt + 1) * P],
                            rhs=wv_v[:, j],
                            start=(j == 0), stop=(j == JC - 1), perf_mode=DR)
                    nc.vector.tensor_copy(
                        vst_v[:, half, :, i, :],
                        ps[:].rearrange("p (h c) -> p h c", h=H))
                nc.sync.dma_start(kin_t.ap()[:, KHALF:],
                                  VST[:, half * VHALF:(half + 1) * VHALF])
                nc.gpsimd.collective_compute(
                    "AllGather", mybir.AluOpType.bypass, replica_groups=groups,
                    ins=[kin_t.ap().opt()], outs=[kout_t.ap().opt()])
                # Q^T for this half's query rows
                for mc in range(CCH):
                    ps = mmps.tile([P, RH], dt.float32, tag="mm", name="qp")
                    for j in range(JC):
                        nc.tensor.matmul(
                            ps[:], lhsT=wq_v[:, j, mc],
                            rhs=ht8c[:, 2 * j:2 * j + 2,
                                     half * RH:(half + 1) * RH],
                            start=(j == 0), stop=(j == JC - 1), perf_mode=DR)
                    nc.vector.tensor_copy(
                        QT8[:, mc * ROWS + half * RH: mc * ROWS + (half + 1) * RH],
                        ps[:])

            def unload_k(li, half):
                """Unload gathered K^T of (layer li, sender half) into
                KT8[li%2] (kts 4rr+2half, +1).  Emitted at the consuming
                phase start so the wait never head-of-line-blocks the
                tails' transposes/ships on the Sync DMA queue."""
                par = li % 2
                kout_t = kout[par][half]
                for rr in range(GROUP):
                    src = kout_t[rr][:, 0:KHALF].rearrange(
                        "p (mc s k) -> p mc s k", mc=CCH, s=2)
                    k0 = (4 * rr + 2 * half) * P
                    dst = ktvs[par][:, :, k0:k0 + 2 * P].rearrange(
                        "p mc (s k) -> p mc s k", s=2)
                    nc.sync.dma_start(dst, src)

            def unload_v(li, half):
                """Unload gathered V of (layer li, sender half) into
                vv[li%2] chunks 2rr+half."""
                par = li % 2
                kout_t = kout[par][half]
                for rr in range(GROUP):
                    srcv = kout_t[rr][:, KHALF:].rearrange(
                        "p (h s c) -> p h s c", h=H, s=2)
                    for hh in range(H):
                        nc.sync.dma_start(
                            vvs[par][:, hh, 2 * rr + half, :, CH:P],
                            srcv[:, hh])

            def score_quad(li, half, h, kts4, col0, esb):
                sub = (h % 2) * CH
                cc_h = h // 2
                sp = scps.tile([P, 4 * RH], dt.float32, tag="sc", name="sc")
                for idx, kt in enumerate(kts4):
                    nc.tensor.matmul(
                        sp[:, idx * RH:(idx + 1) * RH],
                        lhsT=KT8[li % 2][sub:sub + CH,
                                         cc_h * S + kt * P: cc_h * S + (kt + 1) * P],
                        rhs=QT8[sub:sub + CH,
                                cc_h * ROWS + half * RH: cc_h * ROWS + (half + 1) * RH],
                        start=True, stop=True)
                return sp

            def emit_exp(sp, esb, col0):
                nc.scalar.activation(
                    esb[:, col0:col0 + 4 * RH],
                    sp[:], mybir.ActivationFunctionType.Exp,
                    scale=1.0 / np.sqrt(CH))

            def normalize(half, h, av):
                """av = [sums (64 copies) | o^T] for 256 queries."""
                sub = (h % 2) * CH
                cc_h = h // 2
                slot = (h % 2) * ROWS + half * RH
                rf = RECF[0:1, slot:slot + RH]
                nc.vector.reciprocal_approx_fast(rf, av[0:1, :])
                rb = RB[0:P, slot:slot + RH]
                nc.gpsimd.partition_broadcast(rb, rf, channels=P)
                nc.vector.tensor_mul(
                    OSC[sub:sub + CH,
                        cc_h * ROWS + half * RH: cc_h * ROWS + (half + 1) * RH],
                    av[CH:P, :],
                    RB[CH:P, slot:slot + RH])

            # ---- prologue: LN(0) + layer-0 K/V/Q + gathers for both halves
            W = load_w(0)
            tail(-1, 0, W)
            tail(-1, 1, W)

            for li in range(n_layers):
                par = li % 2
                if li + 1 < n_layers:
                    Wn = load_w(li + 1)
                else:
                    Wn = None
                    WG = wpool.tile([P, CCH * C], dt.float8e4, tag="wq")
                    WF = wpool.tile([P, 2 * C], dt.float8e4, tag="wf")
                    nc.sync.dma_start(WG[:], wg_d.ap())
                    nc.sync.dma_start(WF[:], wf_d.ap())
                # even key-chunks for both halves (only needs gather[0])
                unload_k(li, 0)
                esbs = [[None] * H, [None] * H]
                for half in range(2):
                    for h in range(H):
                        esbs[half][h] = epool.tile([P, KT * RH], dt.float8e4,
                                                   tag="esb", name="esb")
                        for kts4, col0 in EQUADS:
                            sp = score_quad(li, half, h, kts4, col0,
                                            esbs[half][h])
                            emit_exp(sp, esbs[half][h], col0)
                # odd key-chunks + AV (one head behind) + tail per half
                unload_k(li, 1)
                unload_v(li, 0)
                unload_v(li, 1)
                # AV accumulation visits even chunks first: the odd-half V
                # gather may still be in flight when the first AVs run
                AVORD = [0, 2, 4, 6, 1, 3, 5, 7]
                for half in range(2):
                    avs = [None] * H
                    for h in range(H):
                        if h > 0:
                            avs[h - 1] = avps.tile([P, RH], dt.float32,
                                                   tag="avp", name="avp")
                        for q, (kts4, col0) in enumerate(OQUADS):
                            sp = score_quad(li, half, h, kts4, col0,
                                            esbs[half][h])
                            if h > 0:
                                pesb = esbs[half][h - 1]
                                for u in range(4):
                                    i_av = 4 * q + u
                                    ck = AVORD[i_av]
                                    off = esb_off(ck)
                                    nc.tensor.matmul(
                                        avs[h - 1][:],
                                        lhsT=vvs[par][:, h - 1, ck],
                                        rhs=pesb[:, off:off + 2 * RH].rearrange(
                                            "p (s n) -> p s n", s=2),
                                        start=(i_av == 0), stop=(i_av == NCK - 1),
                                        perf_mode=DR)
                            emit_exp(sp, esbs[half][h], col0)
                        if h > 0:
                            normalize(half, h - 1, avs[h - 1])
                    avs[H - 1] = avps.tile([P, RH], dt.float32,
                                           tag="avp", name="avp")
                    for i_av, ck in enumerate(AVORD):
                        off = esb_off(ck)
                        nc.tensor.matmul(
                            avs[H - 1][:],
                            lhsT=vvs[par][:, H - 1, ck],
                            rhs=esbs[half][H - 1][:, off:off + 2 * RH].rearrange(
                                "p (s n) -> p s n", s=2),
                            start=(i_av == 0), stop=(i_av == NCK - 1),
                            perf_mode=DR)
                    normalize(half, H - 1, avs[H - 1])
                    tail(li, half, Wn, wo_v=W[3])
                W = Wn

            # ---- FFN epilogue (LN(4) done in the last tails) ----
            wg_v = WG[:].rearrange("p (j mc s m) -> p j mc s m", j=JC, mc=CCH, s=2)
            wf_v = WF[:].rearrange("p (s n) -> p s n", s=2)
            if use_bg:
                BG = gpool.tile([P, CCH], dt.float32, tag="bg")
                nc.sync.dma_start(BG[:], bg_d.ap())
            AGT = small.tile([P, 2 * ROWS], dt.bfloat16, tag="ffa")
            GGT = small.tile([P, 2 * ROWS], dt.bfloat16, tag="ffg")
            for mg in (0, 2, 1, 3):
                ps = mmps.tile([P, ROWS], dt.float32, tag="mm", name="gp")
                for j in range(JC):
                    nc.tensor.matmul(
                        ps[:], lhsT=wg_v[:, j, mg], rhs=ht8c[:, 2 * j:2 * j + 2, :],
                        start=(j == 0), stop=(j == JC - 1), perf_mode=DR)
                dst = (AGT if mg < 2 else GGT)[:, (mg % 2) * ROWS:(mg % 2 + 1) * ROWS]
                if use_bg:
                    nc.vector.tensor_scalar(dst, ps[:], BG[:, mg:mg + 1], None,
                                            mybir.AluOpType.add)
                else:
                    nc.vector.tensor_copy(dst, ps[:])
            K1 = 0.7978845608
            for j in range(2):
                ga = GGT[:, j * ROWS:(j + 1) * ROWS]
                aa = AGT[:, j * ROWS:(j + 1) * ROWS]
                SQ = small.tile([P, ROWS], dt.float32, tag="sq")
                WT = small.tile([P, ROWS], dt.float32, tag="wt")
                VV = small.tile([P, ROWS], dt.float32, tag="vv")
                TT = small.tile([P, ROWS], dt.float32, tag="tt")
                HT2 = small.tile([P, ROWS], dt.bfloat16, tag="ht2")
                PP = small.tile([P, ROWS], dt.bfloat16, tag="pp")
                nc.scalar.activation(SQ[:], ga, mybir.ActivationFunctionType.Square)
                nc.vector.tensor_scalar(WT[:], SQ[:], K1 * 0.044715, K1,
                                        mybir.AluOpType.mult, mybir.AluOpType.add)
                nc.vector.tensor_mul(VV[:], ga, WT[:])
                nc.scalar.activation(TT[:], VV[:], mybir.ActivationFunctionType.Tanh)
                nc.vector.tensor_scalar(HT2[:], TT[:], 0.5, 0.5,
                                        mybir.AluOpType.mult, mybir.AluOpType.add)
                nc.vector.tensor_mul(PP[:], aa, ga)
                nc.vector.tensor_mul(FFSB[:, j * ROWS:(j + 1) * ROWS], PP[:], HT2[:])
            if use_bf:
                BF = gpool.tile([P, C], dt.float32, tag="bf")
                nc.sync.dma_start(BF[:], bf_d.ap())
            for rt in range(RT):
                ps = mmps.tile([P, C], dt.float32, tag="mm", name="fp")
                nc.tensor.matmul(
                    ps[:],
                    lhsT=ff_v[:, :, rt * P:(rt + 1) * P],
                    rhs=wf_v[:],
                    start=True, stop=True, perf_mode=DR)
                OUT = small.tile([P, C], dt.float32, tag="out")
                nc.vector.tensor_add(OUT[:], ps[:], XSB[:, rt * C:(rt + 1) * C])
                if use_bf:
                    nc.vector.tensor_add(OUT[:], OUT[:], BF[:])
                nc.sync.dma_start(y_d.ap()[rt * P:(rt + 1) * P, :], OUT[:])

    nc.compile()
    return nc


def _prepare(x, ln_gamma, ln_beta, Wq, Wk, Wv, Wo, bo, Wg, bg, Wf, bf):
    """Host-side packing: returns (flags, per-core input maps)."""
    x = np.asarray(x, np.float32)
    ln_gamma = np.asarray(ln_gamma, np.float32)
    ln_beta = np.asarray(ln_beta, np.float32)
    Wq, Wk, Wv, Wo = (np.asarray(w, np.float32) for w in (Wq, Wk, Wv, Wo))
    bo = np.asarray(bo, np.float32)
    Wg, Wf = np.asarray(Wg, np.float32), np.asarray(Wf, np.float32)
    bg, bf = np.asarray(bg, np.float32), np.asarray(bf, np.float32)

    use_gamma = not np.all(ln_gamma == 1.0)
    use_beta = not np.all(ln_beta == 0.0)
    use_bo = not np.all(bo == 0.0)
    use_bg = not np.all(bg == 0.0)
    use_bf = not np.all(bf == 0.0)
    flags = (use_gamma, use_beta, use_bo, use_bg, use_bf)

    base = {
        "wq": np.stack([_pack_T(_f8(Wq[l])) for l in range(L)]),
        "wk": np.stack([_pack_T(_f8(Wk[l])) for l in range(L)]),
        "wv": np.stack([_pack_CP(_f8(Wv[l])) for l in range(L)]),
        "wo": np.stack([_pack_CP(_f8(Wo[l])) for l in range(L)]),
        "wg": _pack_T(_f8(Wg)),
        "wf": _pack_WF(_f8(Wf)),
    }
    if use_gamma:
        base["gam"] = np.ascontiguousarray(
            np.broadcast_to(ln_gamma[:, None, :], (L + 1, P, C))).astype(np.float32)
    if use_beta:
        base["bet"] = np.ascontiguousarray(
            np.broadcast_to(ln_beta[:, None, :], (L + 1, P, C))).astype(np.float32)
    if use_bo:
        base["bob"] = np.ascontiguousarray(
            np.broadcast_to(bo[:, None, :], (L, P, C))).astype(np.float32)
    if use_bg:
        base["bgc"] = np.ascontiguousarray(bg.reshape(CCH, P).T).astype(np.float32)
    if use_bf:
        base["bfb"] = np.ascontiguousarray(
            np.broadcast_to(bf[None, :], (P, C))).astype(np.float32)

    xf = x.reshape(B * S, C)
    in_maps = []
    for c in range(N_CORES):
        m = dict(base)
        xc = xf[c * ROWS:(c + 1) * ROWS]
        m["x"] = np.ascontiguousarray(
            xc.reshape(RT, P, C).transpose(1, 0, 2).reshape(P, RT * C))
        in_maps.append(m)
    return flags, in_maps


def kernel(x, ln_gamma, ln_beta, Wq, Wk, Wv, Wo, bo, Wg, bg, Wf, bf):
    flags, in_maps = _prepare(x, ln_gamma, ln_beta, Wq, Wk, Wv, Wo, bo,
                              Wg, bg, Wf, bf)
    if flags not in _CACHE:
        _CACHE[flags] = _build(flags)
    nc = _CACHE[flags]

    from concourse.bass_utils import run_bass_kernel_spmd
    res = run_bass_kernel_spmd(nc, in_maps, core_ids=list(range(N_CORES)))
    out = np.concatenate([res.results[c]["y"] for c in range(N_CORES)], axis=0)
    return out.reshape(B, S, C).astype(np.float32)
